# revision 4
# baseline (speedup 1.0000x reference)
"""NoisyNet dense layer (training mode) on 8 TRN2 NeuronCores.

out[b,u] = x @ W_mu + eps_out * ((x*eps_in) @ W_sigma) + bias_mu + bias_sigma*eps_out

Sharding: data-parallel over batch (4096 -> 512 rows/core), weights/biases
replicated. On-device math runs in a transposed layout ([D,B]/[U,B]) so the
contraction dim D lands on SBUF partitions; the host does the (free)
transposes, bf16 casts and the final gather.

Schedule: weights stream on the sync HWDGE queue, x/eps_in on the scalar
HWDGE queue (x first, small leading chunk so the PE starts early),
eps_out/bias/output on the gpsimd SWDGE queue. The mean-term matmuls run
two U-tiles ahead of the noise-term matmuls so the noise path never waits
on the z = x*eps_in elementwise product during startup.
"""

import numpy as np
import ml_dtypes

import concourse.bacc as bacc
import concourse.mybir as mybir
import concourse.tile as tile
from concourse.bass_utils import run_bass_kernel_spmd

N_CORES = 8
B, D, U = 4096, 2048, 2048
BL = B // N_CORES          # 512 batch rows per core
P = 128                    # partitions
KT = D // P                # 16 contraction tiles
UT = U // P                # 16 output tiles
KC = 4                     # k-tiles per activation DMA chunk
NCH = KT // KC             # 4 chunks
BF16 = mybir.dt.bfloat16
FP32 = mybir.dt.float32

_NBF = ml_dtypes.bfloat16

_cached = None


def _build():
    nc = bacc.Bacc("TRN2", target_bir_lowering=False, debug=False)

    # activations laid out [P, KT, BL]: partition p holds d = k*128+p
    xT = nc.declare_dram_parameter("xT", [P, KT, BL], BF16, isOutput=False)
    eiT = nc.declare_dram_parameter("eiT", [P, KT, BL], BF16, isOutput=False)
    eoT = nc.declare_dram_parameter("eoT", [P, UT, BL], BF16, isOutput=False)
    wmu = nc.declare_dram_parameter("wmu", [UT, P, KT * P], BF16, isOutput=False)
    wsg = nc.declare_dram_parameter("wsg", [UT, P, KT * P], BF16, isOutput=False)
    bmu = nc.declare_dram_parameter("bmu", [P, UT], FP32, isOutput=False)
    bsg = nc.declare_dram_parameter("bsg", [P, UT], FP32, isOutput=False)
    outT = nc.declare_dram_parameter("outT", [UT, P, BL], FP32, isOutput=True)

    SKEW = 2  # mean k-loops run this many U-tiles ahead of noise k-loops

    with tile.TileContext(nc) as tc:
        with (
            tc.tile_pool(name="acts", bufs=1) as acts,
            tc.tile_pool(name="w", bufs=4) as wp,
            tc.tile_pool(name="bias", bufs=1) as bp,
            tc.tile_pool(name="psum", bufs=4, space="PSUM") as pp,
            tc.tile_pool(name="tmp", bufs=2) as tp,
            tc.tile_pool(name="out", bufs=3) as op,
        ):
            # Weight stream (sync queue). First mu chunk is split so the
            # very first matmuls have a small fast DMA to wait on.
            wm_tiles = {}
            ws_tiles = {}

            def fetch_wm(u, split=False):
                wm = wp.tile([P, KT * P], BF16, tag="wm")
                if split:
                    nc.sync.dma_start(wm[:, :KC * P], wmu[u][:, :KC * P])
                    nc.sync.dma_start(wm[:, KC * P:], wmu[u][:, KC * P:])
                else:
                    nc.sync.dma_start(wm[:], wmu[u])
                wm_tiles[u] = wm

            def fetch_ws(u):
                ws = wp.tile([P, KT * P], BF16, tag="ws")
                nc.sync.dma_start(ws[:], wsg[u])
                ws_tiles[u] = ws

            fetch_wm(0, split=True)
            fetch_wm(1)
            fetch_ws(0)
            fetch_ws(1)

            # Activation stream (scalar queue): all of x first (mean path),
            # then eps_in (noise path). Leading x chunk is a single k-tile.
            x_sb = acts.tile([P, KT, BL], BF16, tag="x")
            ei_sb = acts.tile([P, KT, BL], BF16, tag="ei")
            z_sb = acts.tile([P, KT, BL], BF16, tag="z")
            eo_sb = acts.tile([P, UT, BL], BF16, tag="eo")

            nc.scalar.dma_start(x_sb[:, 0:1, :], xT[:, 0:1, :])
            nc.scalar.dma_start(x_sb[:, 1:KC, :], xT[:, 1:KC, :])
            for c in range(1, NCH):
                s = slice(c * KC, (c + 1) * KC)
                nc.scalar.dma_start(x_sb[:, s, :], xT[:, s, :])
            for c in range(NCH):
                s = slice(c * KC, (c + 1) * KC)
                nc.scalar.dma_start(ei_sb[:, s, :], eiT[:, s, :])
                nc.vector.tensor_mul(z_sb[:, s, :], x_sb[:, s, :], ei_sb[:, s, :])

            # eps_out / biases / outputs ride the gpsimd SWDGE queue.
            bmu_t = bp.tile([P, UT], FP32, tag="bmu")
            nc.gpsimd.dma_start(bmu_t[:], bmu[:])
            bsg_t = bp.tile([P, UT], FP32, tag="bsg")
            nc.gpsimd.dma_start(bsg_t[:], bsg[:])
            for c in range(NCH):
                s = slice(c * KC, (c + 1) * KC)
                nc.gpsimd.dma_start(eo_sb[:, s, :], eoT[:, s, :])

            ps_m = {}

            def mean_kloop(u):
                pm = pp.tile([P, BL], FP32, tag="psm")
                wm = wm_tiles.pop(u)
                for k in range(KT):
                    nc.tensor.matmul(
                        pm[:], wm[:, k * P:(k + 1) * P], x_sb[:, k, :],
                        start=(k == 0), stop=(k == KT - 1),
                    )
                ps_m[u] = pm

            mean_kloop(0)
            mean_kloop(1)

            for u in range(UT):
                # prefetch weights SKEW ahead (wm 0/1 already fetched)
                un = u + SKEW
                if un < UT:
                    fetch_wm(un)
                if u + 1 < UT:
                    fetch_ws(u + 1)

                # noise k-loop for u
                pn = pp.tile([P, BL], FP32, tag="psn")
                ws = ws_tiles.pop(u)
                for k in range(KT):
                    nc.tensor.matmul(
                        pn[:], ws[:, k * P:(k + 1) * P], z_sb[:, k, :],
                        start=(k == 0), stop=(k == KT - 1),
                    )
                # mean k-loop for u+SKEW
                if u + SKEW < UT:
                    mean_kloop(u + SKEW)

                pm = ps_m.pop(u)
                t_n = tp.tile([P, BL], FP32, tag="tn")
                nc.scalar.add(t_n[:], pn[:], bsg_t[:, u:u + 1])
                t_m = tp.tile([P, BL], FP32, tag="tm")
                nc.scalar.add(t_m[:], pm[:], bmu_t[:, u:u + 1])
                pr = tp.tile([P, BL], FP32, tag="pr")
                nc.vector.tensor_mul(pr[:], t_n[:], eo_sb[:, u, :])
                o = op.tile([P, BL], FP32, tag="o")
                nc.vector.tensor_add(o[:], pr[:], t_m[:])
                nc.gpsimd.dma_start(outT[u], o[:])

    nc.compile()
    return nc


def _get_nc():
    global _cached
    if _cached is None:
        _cached = _build()
    return _cached


def kernel(x, weight_mu, weight_sigma, bias_mu, bias_sigma, eps_in, eps_out,
           _trace=False):
    nc = _get_nc()

    # Host-side layout prep (transposes + bf16 casts only; no layer math).
    def to_pkb(a):  # [B, D] -> per-core [P, KT, BL] (partition p holds k*128+p)
        a = np.ascontiguousarray(a.astype(_NBF))
        return [
            np.ascontiguousarray(
                a[c * BL:(c + 1) * BL].T.reshape(KT, P, BL).transpose(1, 0, 2))
            for c in range(N_CORES)
        ]

    xs = to_pkb(x)
    eis = to_pkb(eps_in)
    eos = to_pkb(eps_out)  # same transform, u in place of k

    def w_blocks(w):  # [D, U] -> [UT, P(d within block), KT*P] bf16
        wb = w.astype(_NBF).reshape(KT, P, UT, P).transpose(2, 1, 0, 3)
        return np.ascontiguousarray(wb.reshape(UT, P, KT * P))

    wmu_h = w_blocks(weight_mu)
    wsg_h = w_blocks(weight_sigma)
    bmu_h = np.ascontiguousarray(bias_mu.astype(np.float32).reshape(UT, P).T)
    bsg_h = np.ascontiguousarray(bias_sigma.astype(np.float32).reshape(UT, P).T)

    in_maps = [
        {
            "xT": xs[c],
            "eiT": eis[c],
            "eoT": eos[c],
            "wmu": wmu_h,
            "wsg": wsg_h,
            "bmu": bmu_h,
            "bsg": bsg_h,
        }
        for c in range(N_CORES)
    ]

    res = run_bass_kernel_spmd(nc, in_maps, core_ids=list(range(N_CORES)),
                               trace=_trace)
    kernel.last_result = res

    out = np.empty((B, U), dtype=np.float32)
    for c in range(N_CORES):
        oc = res.results[c]["outT"]  # [UT, P, BL]
        out[c * BL:(c + 1) * BL] = oc.transpose(2, 0, 1).reshape(BL, U)
    return out


# revision 6
# speedup vs baseline: 1.0590x; 1.0590x over previous
"""NoisyNet dense layer (training mode) on 8 TRN2 NeuronCores.

out[b,u] = x @ W_mu + eps_out * ((x*eps_in) @ W_sigma) + bias_mu + bias_sigma*eps_out

Sharding: data-parallel over batch (4096 -> 512 rows/core), weights/biases
replicated. On-device math runs in a transposed layout ([D,B]/[U,B]) so the
contraction dim D lands on SBUF partitions; the host does the (free)
transposes, bf16 casts and the final gather.

Schedule: weights stream on the sync HWDGE queue, x/eps_in on the scalar
HWDGE queue (x first, small leading chunk so the PE starts early),
eps_out/bias/output on the gpsimd SWDGE queue. The mean-term matmuls run
two U-tiles ahead of the noise-term matmuls so the noise path never waits
on the z = x*eps_in elementwise product during startup.
"""

import numpy as np
import ml_dtypes

import concourse.bacc as bacc
import concourse.mybir as mybir
import concourse.tile as tile
from concourse.bass_utils import run_bass_kernel_spmd

N_CORES = 8
B, D, U = 4096, 2048, 2048
BL = B // N_CORES          # 512 batch rows per core
P = 128                    # partitions
KT = D // P                # 16 contraction tiles
UT = U // P                # 16 output tiles
KC = 4                     # k-tiles per activation DMA chunk
NCH = KT // KC             # 4 chunks
BF16 = mybir.dt.bfloat16
FP32 = mybir.dt.float32

_NBF = ml_dtypes.bfloat16

_cached = None


def _build():
    nc = bacc.Bacc("TRN2", target_bir_lowering=False, debug=False)

    # activations laid out [P, KT, BL]: partition p holds d = k*128+p
    xT = nc.declare_dram_parameter("xT", [P, KT, BL], BF16, isOutput=False)
    eiT = nc.declare_dram_parameter("eiT", [P, KT, BL], BF16, isOutput=False)
    eoT = nc.declare_dram_parameter("eoT", [P, UT, BL], BF16, isOutput=False)
    wmu = nc.declare_dram_parameter("wmu", [UT, P, KT * P], BF16, isOutput=False)
    wsg = nc.declare_dram_parameter("wsg", [UT, P, KT * P], BF16, isOutput=False)
    bmu = nc.declare_dram_parameter("bmu", [P, UT], FP32, isOutput=False)
    bsg = nc.declare_dram_parameter("bsg", [P, UT], FP32, isOutput=False)
    outT = nc.declare_dram_parameter("outT", [UT, P, BL], FP32, isOutput=True)

    SKEW = 3  # mean k-loops run this many U-tiles ahead of noise k-loops

    with tile.TileContext(nc) as tc:
        with (
            tc.tile_pool(name="acts", bufs=1) as acts,
            tc.tile_pool(name="w", bufs=4) as wp,
            tc.tile_pool(name="bias", bufs=1) as bp,
            tc.tile_pool(name="psum", bufs=4, space="PSUM") as pp,
            tc.tile_pool(name="psumn", bufs=3, space="PSUM") as ppn,
            tc.tile_pool(name="tmp", bufs=2) as tp,
            tc.tile_pool(name="out", bufs=3) as op,
        ):
            # HAM warm-up: a run of matmuls on zeroed SBUF keeps the PE
            # activity monitor busy during the initial DMA wait so the first
            # real matmuls run at 2.4 GHz instead of 1.2 GHz.
            warm_in = bp.tile([P, BL], BF16, tag="warmin")
            nc.gpsimd.memset(warm_in[:], 0.0)
            warm_ps = ppn.tile([P, BL], FP32, tag="psn")
            for _ in range(10):
                nc.tensor.matmul(warm_ps[:], warm_in[:, :P], warm_in[:])

            # Weight stream (sync queue). First mu chunk is split so the
            # very first matmuls have a small fast DMA to wait on.
            wm_tiles = {}
            ws_tiles = {}

            def fetch_wm(u, split=False):
                wm = wp.tile([P, KT * P], BF16, tag="wm")
                if split:
                    nc.sync.dma_start(wm[:, :KC * P], wmu[u][:, :KC * P])
                    nc.sync.dma_start(wm[:, KC * P:], wmu[u][:, KC * P:])
                else:
                    nc.sync.dma_start(wm[:], wmu[u])
                wm_tiles[u] = wm

            def fetch_ws(u):
                ws = wp.tile([P, KT * P], BF16, tag="ws")
                nc.sync.dma_start(ws[:], wsg[u])
                ws_tiles[u] = ws

            fetch_wm(0, split=True)
            fetch_wm(1)
            fetch_ws(0)
            fetch_ws(1)

            # Activation stream (scalar queue): x and eps_in interleaved
            # (leading 1-k-tile slices so the pipeline starts fast), then
            # eps_out last — FIFO order delays it past the startup crunch.
            x_sb = acts.tile([P, KT, BL], BF16, tag="x")
            ei_sb = acts.tile([P, KT, BL], BF16, tag="ei")
            z_sb = acts.tile([P, KT, BL], BF16, tag="z")
            eo_sb = acts.tile([P, UT, BL], BF16, tag="eo")

            nc.scalar.dma_start(x_sb[:, 0:1, :], xT[:, 0:1, :])
            nc.scalar.dma_start(ei_sb[:, 0:1, :], eiT[:, 0:1, :])
            nc.vector.tensor_mul(z_sb[:, 0:1, :], x_sb[:, 0:1, :], ei_sb[:, 0:1, :])
            nc.scalar.dma_start(x_sb[:, 1:KC, :], xT[:, 1:KC, :])
            nc.scalar.dma_start(ei_sb[:, 1:KC, :], eiT[:, 1:KC, :])
            nc.vector.tensor_mul(z_sb[:, 1:KC, :], x_sb[:, 1:KC, :], ei_sb[:, 1:KC, :])
            for c in range(1, NCH):
                s = slice(c * KC, (c + 1) * KC)
                nc.scalar.dma_start(x_sb[:, s, :], xT[:, s, :])
                nc.scalar.dma_start(ei_sb[:, s, :], eiT[:, s, :])
                nc.vector.tensor_mul(z_sb[:, s, :], x_sb[:, s, :], ei_sb[:, s, :])
            for c in range(NCH):
                s = slice(c * KC, (c + 1) * KC)
                nc.scalar.dma_start(eo_sb[:, s, :], eoT[:, s, :])

            # biases (tiny, early) and outputs ride the gpsimd SWDGE queue.
            bmu_t = bp.tile([P, UT], FP32, tag="bmu")
            nc.gpsimd.dma_start(bmu_t[:], bmu[:])
            bsg_t = bp.tile([P, UT], FP32, tag="bsg")
            nc.gpsimd.dma_start(bsg_t[:], bsg[:])

            ps_m = {}

            def mean_kloop(u):
                pm = pp.tile([P, BL], FP32, tag="psm")
                wm = wm_tiles.pop(u)
                for k in range(KT):
                    nc.tensor.matmul(
                        pm[:], wm[:, k * P:(k + 1) * P], x_sb[:, k, :],
                        start=(k == 0), stop=(k == KT - 1),
                    )
                ps_m[u] = pm

            for i in range(SKEW):
                if i >= 2:
                    fetch_wm(i)
                mean_kloop(i)

            for u in range(UT):
                # prefetch weights SKEW ahead (wm 0/1 already fetched)
                un = u + SKEW
                if un < UT:
                    fetch_wm(un)
                if u + 1 < UT:
                    fetch_ws(u + 1)

                # noise k-loop for u
                pn = ppn.tile([P, BL], FP32, tag="psn")
                ws = ws_tiles.pop(u)
                for k in range(KT):
                    nc.tensor.matmul(
                        pn[:], ws[:, k * P:(k + 1) * P], z_sb[:, k, :],
                        start=(k == 0), stop=(k == KT - 1),
                    )
                # mean k-loop for u+SKEW
                if u + SKEW < UT:
                    mean_kloop(u + SKEW)

                pm = ps_m.pop(u)
                t_n = tp.tile([P, BL], FP32, tag="tn")
                nc.scalar.add(t_n[:], pn[:], bsg_t[:, u:u + 1])
                t_m = tp.tile([P, BL], FP32, tag="tm")
                nc.scalar.add(t_m[:], pm[:], bmu_t[:, u:u + 1])
                pr = tp.tile([P, BL], FP32, tag="pr")
                nc.vector.tensor_mul(pr[:], t_n[:], eo_sb[:, u, :])
                o = op.tile([P, BL], FP32, tag="o")
                nc.vector.tensor_add(o[:], pr[:], t_m[:])
                nc.gpsimd.dma_start(outT[u], o[:])

    nc.compile()
    return nc


def _get_nc():
    global _cached
    if _cached is None:
        _cached = _build()
    return _cached


def kernel(x, weight_mu, weight_sigma, bias_mu, bias_sigma, eps_in, eps_out,
           _trace=False):
    nc = _get_nc()

    # Host-side layout prep (transposes + bf16 casts only; no layer math).
    def to_pkb(a):  # [B, D] -> per-core [P, KT, BL] (partition p holds k*128+p)
        a = np.ascontiguousarray(a.astype(_NBF))
        return [
            np.ascontiguousarray(
                a[c * BL:(c + 1) * BL].T.reshape(KT, P, BL).transpose(1, 0, 2))
            for c in range(N_CORES)
        ]

    xs = to_pkb(x)
    eis = to_pkb(eps_in)
    eos = to_pkb(eps_out)  # same transform, u in place of k

    def w_blocks(w):  # [D, U] -> [UT, P(d within block), KT*P] bf16
        wb = w.astype(_NBF).reshape(KT, P, UT, P).transpose(2, 1, 0, 3)
        return np.ascontiguousarray(wb.reshape(UT, P, KT * P))

    wmu_h = w_blocks(weight_mu)
    wsg_h = w_blocks(weight_sigma)
    bmu_h = np.ascontiguousarray(bias_mu.astype(np.float32).reshape(UT, P).T)
    bsg_h = np.ascontiguousarray(bias_sigma.astype(np.float32).reshape(UT, P).T)

    in_maps = [
        {
            "xT": xs[c],
            "eiT": eis[c],
            "eoT": eos[c],
            "wmu": wmu_h,
            "wsg": wsg_h,
            "bmu": bmu_h,
            "bsg": bsg_h,
        }
        for c in range(N_CORES)
    ]

    res = run_bass_kernel_spmd(nc, in_maps, core_ids=list(range(N_CORES)),
                               trace=_trace)
    kernel.last_result = res

    out = np.empty((B, U), dtype=np.float32)
    for c in range(N_CORES):
        oc = res.results[c]["outT"]  # [UT, P, BL]
        out[c * BL:(c + 1) * BL] = oc.transpose(2, 0, 1).reshape(BL, U)
    return out


# revision 7
# speedup vs baseline: 1.1186x; 1.0563x over previous
"""NoisyNet dense layer (training mode) on 8 TRN2 NeuronCores.

out[b,u] = x @ W_mu + eps_out * ((x*eps_in) @ W_sigma) + bias_mu + bias_sigma*eps_out

Sharding: data-parallel over batch (4096 -> 512 rows/core), weights/biases
replicated. On-device math runs in a transposed layout ([D,B]/[U,B]) so the
contraction dim D lands on SBUF partitions; the host does the (free)
transposes, bf16 casts and the final gather.

Two-phase schedule to flatten DMA demand: phase 1 runs all 16 mean-term
k-loops (needs only x + the W_mu stream early; each mean PSUM drains to
SBUF through the bias-add ACT op), while eps_in/W_sigma/eps_out stream in
the background. Phase 2 runs the 16 noise-term k-loops and the final
elementwise combine. The PE never waits on more than ~0.25 MB at start.
"""

import numpy as np
import ml_dtypes

import concourse.bacc as bacc
import concourse.mybir as mybir
import concourse.tile as tile
from concourse.bass_utils import run_bass_kernel_spmd

N_CORES = 8
B, D, U = 4096, 2048, 2048
BL = B // N_CORES          # 512 batch rows per core
P = 128                    # partitions
KT = D // P                # 16 contraction tiles
UT = U // P                # 16 output tiles
KC = 4                     # k-tiles per activation DMA chunk
NCH = KT // KC             # 4 chunks
BF16 = mybir.dt.bfloat16
FP32 = mybir.dt.float32

_NBF = ml_dtypes.bfloat16

_cached = None


def _build():
    nc = bacc.Bacc("TRN2", target_bir_lowering=False, debug=False)

    # activations laid out [P, KT, BL]: partition p holds d = k*128+p
    xT = nc.declare_dram_parameter("xT", [P, KT, BL], BF16, isOutput=False)
    eiT = nc.declare_dram_parameter("eiT", [P, KT, BL], BF16, isOutput=False)
    eoT = nc.declare_dram_parameter("eoT", [P, UT, BL], BF16, isOutput=False)
    wmu = nc.declare_dram_parameter("wmu", [UT, P, KT * P], BF16, isOutput=False)
    wsg = nc.declare_dram_parameter("wsg", [UT, P, KT * P], BF16, isOutput=False)
    bmu = nc.declare_dram_parameter("bmu", [P, UT], FP32, isOutput=False)
    bsg = nc.declare_dram_parameter("bsg", [P, UT], FP32, isOutput=False)
    outT = nc.declare_dram_parameter("outT", [UT, P, BL], FP32, isOutput=True)

    with tile.TileContext(nc) as tc:
        with (
            tc.tile_pool(name="acts", bufs=1) as acts,
            tc.tile_pool(name="w", bufs=4) as wp,
            tc.tile_pool(name="bias", bufs=1) as bp,
            tc.tile_pool(name="psum", bufs=4, space="PSUM") as pp,
            tc.tile_pool(name="psumn", bufs=3, space="PSUM") as ppn,
            tc.tile_pool(name="mean", bufs=1) as mp,
            tc.tile_pool(name="tmp", bufs=2) as tp,
            tc.tile_pool(name="out", bufs=3) as op,
        ):
            # HAM warm-up: matmuls on zeroed SBUF during the initial DMA wait
            # so the first real matmuls run at 2.4 GHz.
            warm_in = bp.tile([P, BL], BF16, tag="warmin")
            nc.gpsimd.memset(warm_in[:], 0.0)
            warm_ps = ppn.tile([P, BL], FP32, tag="psn")
            for _ in range(6):
                nc.tensor.matmul(warm_ps[:], warm_in[:, :P], warm_in[:])

            # Weight stream (sync queue): all W_mu first, then all W_sigma.
            # First mu chunk split small so the first matmul starts fast.
            wm_tiles = {}
            ws_tiles = {}

            def fetch_wm(u, split=False):
                wm = wp.tile([P, KT * P], BF16, tag="wm")
                if split:
                    nc.sync.dma_start(wm[:, :KC * P], wmu[u][:, :KC * P])
                    nc.sync.dma_start(wm[:, KC * P:], wmu[u][:, KC * P:])
                else:
                    nc.sync.dma_start(wm[:], wmu[u])
                wm_tiles[u] = wm

            def fetch_ws(u):
                ws = wp.tile([P, KT * P], BF16, tag="ws")
                nc.sync.dma_start(ws[:], wsg[u])
                ws_tiles[u] = ws

            fetch_wm(0, split=True)
            fetch_wm(1)
            fetch_wm(2)

            # Activation stream (scalar queue): x chunks first (phase-1
            # critical), then eps_in (phase 2), then eps_out (phase-2 tail).
            x_sb = acts.tile([P, KT, BL], BF16, tag="x")
            ei_sb = acts.tile([P, KT, BL], BF16, tag="ei")
            z_sb = acts.tile([P, KT, BL], BF16, tag="z")
            eo_sb = acts.tile([P, UT, BL], BF16, tag="eo")

            nc.scalar.dma_start(x_sb[:, 0:1, :], xT[:, 0:1, :])
            nc.scalar.dma_start(x_sb[:, 1:KC, :], xT[:, 1:KC, :])
            for c in range(1, NCH):
                s = slice(c * KC, (c + 1) * KC)
                nc.scalar.dma_start(x_sb[:, s, :], xT[:, s, :])
            for c in range(NCH):
                s = slice(c * KC, (c + 1) * KC)
                nc.scalar.dma_start(ei_sb[:, s, :], eiT[:, s, :])
                nc.vector.tensor_mul(z_sb[:, s, :], x_sb[:, s, :], ei_sb[:, s, :])
            for c in range(NCH):
                s = slice(c * KC, (c + 1) * KC)
                nc.scalar.dma_start(eo_sb[:, s, :], eoT[:, s, :])

            # biases (tiny) on the gpsimd SWDGE queue, early.
            bmu_t = bp.tile([P, UT], FP32, tag="bmu")
            nc.gpsimd.dma_start(bmu_t[:], bmu[:])
            bsg_t = bp.tile([P, UT], FP32, tag="bsg")
            nc.gpsimd.dma_start(bsg_t[:], bsg[:])

            # ---- Phase 1: mean terms. t_m[u] = W_mu[u].T @ x + bias_mu[u] ----
            t_m = []
            for u in range(UT):
                if u + 3 < UT:
                    fetch_wm(u + 3)
                elif u + 3 == UT:
                    for uu in range(2):
                        fetch_ws(uu)
                wm = wm_tiles.pop(u)
                pm = pp.tile([P, BL], FP32, tag="psm")
                for k in range(KT):
                    nc.tensor.matmul(
                        pm[:], wm[:, k * P:(k + 1) * P], x_sb[:, k, :],
                        start=(k == 0), stop=(k == KT - 1),
                    )
                tm = mp.tile([P, BL], FP32, tag=f"tm{u}")
                nc.scalar.add(tm[:], pm[:], bmu_t[:, u:u + 1])
                t_m.append(tm)

            # ---- Phase 2: noise terms + combine ----
            for u in range(UT):
                un = u + 2
                if 2 <= un < UT:
                    fetch_ws(un)
                ws = ws_tiles.pop(u)
                last = (u == UT - 1)
                # last tile: split batch in halves so the epilogue pipelines
                # with the final matmuls instead of serializing after them.
                halves = (0, BL // 2, BL) if last else (0, BL)
                for h in range(len(halves) - 1):
                    lo, hi = halves[h], halves[h + 1]
                    pn = ppn.tile([P, hi - lo], FP32, tag="psn")
                    for k in range(KT):
                        nc.tensor.matmul(
                            pn[:], ws[:, k * P:(k + 1) * P], z_sb[:, k, lo:hi],
                            start=(k == 0), stop=(k == KT - 1),
                        )
                    t_n = tp.tile([P, hi - lo], FP32, tag="tn")
                    nc.scalar.add(t_n[:], pn[:], bsg_t[:, u:u + 1])
                    pr = tp.tile([P, hi - lo], FP32, tag="pr")
                    nc.vector.tensor_mul(pr[:], t_n[:], eo_sb[:, u, lo:hi])
                    o = op.tile([P, hi - lo], FP32, tag="o")
                    nc.vector.tensor_add(o[:], pr[:], t_m[u][:, lo:hi])
                    nc.gpsimd.dma_start(outT[u][:, lo:hi], o[:])

    nc.compile()
    return nc


def _get_nc():
    global _cached
    if _cached is None:
        _cached = _build()
    return _cached


def kernel(x, weight_mu, weight_sigma, bias_mu, bias_sigma, eps_in, eps_out,
           _trace=False):
    nc = _get_nc()

    # Host-side layout prep (transposes + bf16 casts only; no layer math).
    def to_pkb(a):  # [B, D] -> per-core [P, KT, BL] (partition p holds k*128+p)
        a = np.ascontiguousarray(a.astype(_NBF))
        return [
            np.ascontiguousarray(
                a[c * BL:(c + 1) * BL].T.reshape(KT, P, BL).transpose(1, 0, 2))
            for c in range(N_CORES)
        ]

    xs = to_pkb(x)
    eis = to_pkb(eps_in)
    eos = to_pkb(eps_out)  # same transform, u in place of k

    def w_blocks(w):  # [D, U] -> [UT, P(d within block), KT*P] bf16
        wb = w.astype(_NBF).reshape(KT, P, UT, P).transpose(2, 1, 0, 3)
        return np.ascontiguousarray(wb.reshape(UT, P, KT * P))

    wmu_h = w_blocks(weight_mu)
    wsg_h = w_blocks(weight_sigma)
    bmu_h = np.ascontiguousarray(bias_mu.astype(np.float32).reshape(UT, P).T)
    bsg_h = np.ascontiguousarray(bias_sigma.astype(np.float32).reshape(UT, P).T)

    in_maps = [
        {
            "xT": xs[c],
            "eiT": eis[c],
            "eoT": eos[c],
            "wmu": wmu_h,
            "wsg": wsg_h,
            "bmu": bmu_h,
            "bsg": bsg_h,
        }
        for c in range(N_CORES)
    ]

    res = run_bass_kernel_spmd(nc, in_maps, core_ids=list(range(N_CORES)),
                               trace=_trace)
    kernel.last_result = res

    out = np.empty((B, U), dtype=np.float32)
    for c in range(N_CORES):
        oc = res.results[c]["outT"]  # [UT, P, BL]
        out[c * BL:(c + 1) * BL] = oc.transpose(2, 0, 1).reshape(BL, U)
    return out
